# revision 29
# baseline (speedup 1.0000x reference)
"""Trainium2 Bass kernel for nn_FactorGraphGRU (N=8192, H=64, 8 NeuronCores).

Strategy (memory-bound): row-shard the outputs across 8 cores (1024 each).
Each core streams transposed adjacency shards once from HBM in bf16:

  posn  [N, 1024] bf16  host-built positive mask of node_adj^T (exact 0/1)
  eat   [N, 1024] bf16  edge_adj^T values (bf16 round ~0.4%, tolerance 2e-2)

Per 128-row block the tensor engine runs 4 matmul passes against a
stationary [h_hi | h_lo] bf16 tile (hi/lo split keeps the attention-score
exponents accurate): P (node mask), R=relu(eat), Nm=min(eat,0), count
(pos_e vs ones).  relu on ACT, min/is_gt on DVE (bf16 fast modes); the
GPSIMD engine is never used (its elementwise path measured ~20x slower).

All downstream algebra is folded into host-precomputed stationaries:
  - M = sum_h - h_i - P is eliminated (coefficients on P/h + bias consts)
  - hi/lo recombine is folded into every consumer stationary ([W; W])
The tail runs in the [64, ROWS] transposed layout (this toolchain cannot
encode matmul outputs at a non-zero PSUM base partition).  The GAT softmax
collapses to the two-value form: es = a_p*(W^T R) + a_m*(W^T Nm), with
Z = cp*(wp-wm) + (N-1)*wm from the streamed positive-count row.
"""

import numpy as np
from contextlib import ExitStack

N = 8192
H = 64
NCORES = 8
ROWS = N // NCORES        # 1024 output rows per core
JB = 128                  # contraction block (SBUF partitions)
NJB = N // JB             # 64
NB2 = N // (2 * JB)       # 32 fp8 DoubleRow blocks (256 rows each)
CHUNK = 512               # PSUM bank free size (f32)
NCH = ROWS // CHUNK       # 2
ALPHA = 0.2               # leaky relu slope
DEBUG_DUMP = False        # test hook: dump intermediates as extra outputs
USE_FAST_RECIP = True     # custom-DVE reciprocal (falls back to stock op)


# ---------------------------------------------------------------------------
# walrus workaround: this toolchain accepts at most ONE sync wait per
# instruction; Tile attaches several.  Rewrite the BIR so every extra wait
# rides on its own NoOp carrier right before the instruction.
# ---------------------------------------------------------------------------
def _split_multiwaits(nc):
    import bass_rust
    import concourse.mybir as mybir

    ctr = [0]

    def carrier(engine, wait):
        ctr[0] += 1
        nop = bass_rust.InstNoOp(name=f"WS-{ctr[0]}", engine=engine, ins=[], outs=[])
        nop.sync_info = mybir.SyncInfo(on_wait=[wait], on_update=[])
        return nop

    for fn in nc.m.functions:
        stack = list(fn.blocks)
        while stack:
            bb = stack.pop()
            stack.extend(getattr(bb, "blocks", []) or [])
            out = []
            changed = False
            for inst in bb.instructions:
                si = inst.sync_info
                waits = list(si.on_wait) if si is not None and si.on_wait else []
                if len(waits) > 1:
                    for w in waits[:-1]:
                        out.append(carrier(inst.engine, w))
                    si.on_wait = [waits[-1]]
                    changed = True
                out.append(inst)
            if changed:
                bb.instructions = out
    return nc


def _build_nc():
    import concourse.bass as bass
    import concourse.tile as tile
    from concourse import mybir

    F32 = mybir.dt.float32
    F32R = mybir.dt.float32r
    BF16 = mybir.dt.bfloat16
    FP8 = mybir.dt.float8e4
    AF = mybir.ActivationFunctionType
    OP = mybir.AluOpType

    nc = bass.Bass("TRN2", target_bir_lowering=False, debug=False,
                   num_devices=NCORES)

    # --- DRAM inputs (per-core shards via in_maps) ---
    posn = nc.dram_tensor("posn", [N // 2, 2 * ROWS], BF16, kind="ExternalInput").ap()
    eat = nc.dram_tensor("eat", [N // 2, 2 * ROWS], FP8, kind="ExternalInput").ap()
    h2p_d = nc.dram_tensor("h2p", [JB, N], BF16, kind="ExternalInput").ap()
    h8p_d = nc.dram_tensor("h8p", [JB, N], FP8, kind="ExternalInput").ap()
    ones8_d = nc.dram_tensor("ones8", [JB, 32], FP8, kind="ExternalInput").ap()
    hTp_d = nc.dram_tensor("hTp", [H, ROWS], F32, kind="ExternalInput").ap()
    hTpr_d = nc.dram_tensor("hTpr", [H, ROWS], F32R, kind="ExternalInput").ap()
    WeP_d = nc.dram_tensor("WeP", [2 * H, 3 * H], F32R, kind="ExternalInput").ap()
    Weh_d = nc.dram_tensor("Weh", [H, 4 * H], F32R, kind="ExternalInput").ap()
    WnX_d = nc.dram_tensor("WnX", [H, 3 * H], F32R, kind="ExternalInput").ap()
    Wnh_d = nc.dram_tensor("Wnh", [H, 3 * H], F32R, kind="ExternalInput").ap()
    be4_d = nc.dram_tensor("be4", [H, 4], F32, kind="ExternalInput").ap()
    bn4_d = nc.dram_tensor("bn4", [H, 4], F32, kind="ExternalInput").ap()
    Wg2_d = nc.dram_tensor("Wg2", [2 * H, H], F32R, kind="ExternalInput").ap()
    vecsP_d = nc.dram_tensor("vecsP", [2 * H, 2], F32R, kind="ExternalInput").ap()
    vech_d = nc.dram_tensor("vech", [H, 2], F32R, kind="ExternalInput").ap()
    cbias_d = nc.dram_tensor("cbias", [1, 2], F32, kind="ExternalInput").ap()
    ones1_d = nc.dram_tensor("ones1", [1, H], F32R, kind="ExternalInput").ap()
    d_er_d = nc.dram_tensor("d_er", [1, ROWS], F32R, kind="ExternalInput").ap()
    d_nr_d = nc.dram_tensor("d_nr", [1, ROWS], F32R, kind="ExternalInput").ap()
    out = nc.dram_tensor("out", [H, ROWS], F32, kind="ExternalOutput").ap()
    dbg = {}
    if DEBUG_DUMP:
        for nm, sh in [("d_P", [2 * H, CHUNK]), ("d_ep", [1, ROWS]),
                       ("d_em", [1, ROWS]), ("d_cp", [1, ROWS]),
                       ("d_ap", [1, ROWS]), ("d_am", [1, ROWS]),
                       ("d_spos", [H, ROWS]), ("d_es", [H, ROWS]),
                       ("d_eo", [H, ROWS]), ("d_no", [H, ROWS])]:
            dbg[nm] = nc.dram_tensor(nm, sh, F32, kind="ExternalOutput").ap()

    with tile.TileContext(nc) as tc, ExitStack() as ctx:
        # --- pools ---
        pnp = ctx.enter_context(tc.tile_pool(name="pnp", bufs=3))
        eap = ctx.enter_context(tc.tile_pool(name="eap", bufs=6))
        var = ctx.enter_context(tc.tile_pool(name="var", bufs=4))
        small = ctx.enter_context(tc.tile_pool(name="small", bufs=1))
        work = ctx.enter_context(tc.tile_pool(name="work", bufs=1))
        psAcc = tc.alloc_tile_pool(name="psAcc", bufs=1, space="PSUM")

        # --- small persistent inputs ---
        def load_small(src, shape, name, dt=F32):
            t = small.tile(shape, dt, name=name)
            nc.sync.dma_start(t[:], src[:])
            return t

        # h2p/h8p loaded in 8 slices just-in-time (slice q covers blocks
        # [4q, 4q+4); q+1 is issued at block 4q+1, three blocks of lead)
        h2ps = small.tile([JB, N], BF16, name="h2ps")
        h8ps = small.tile([JB, N], FP8, name="h8ps")

        def load_hslices(q):
            qs = slice(q * (N // 8), (q + 1) * (N // 8))
            nc.sync.dma_start(h2ps[:, qs], h2p_d[:, qs])
            nc.sync.dma_start(h8ps[:, qs], h8p_d[:, qs])

        load_hslices(0)
        ones8 = load_small(ones8_d, [JB, 32], "ones8", FP8)

        # --- PSUM accumulators: 6x[128,512] + 2x[1,512] = 8 banks ---
        psP = [psAcc.tile([2 * H, CHUNK], F32, name=f"psP{c}", tag=f"psP{c}")
               for c in range(NCH)]
        psR = [psAcc.tile([2 * H, CHUNK], F32, name=f"psR{c}", tag=f"psR{c}")
               for c in range(NCH)]
        psN = [psAcc.tile([2 * H, CHUNK], F32, name=f"psN{c}", tag=f"psN{c}")
               for c in range(NCH)]
        psC = [psAcc.tile([16, CHUNK], F32, name=f"psC{c}", tag=f"psC{c}")
               for c in range(NCH)]

        # =================== stream: one pass over both adjacencies ========
        # node mask: bf16 128-row tiles; edge: fp8 DoubleRow 256-row blocks
        DR = mybir.MatmulPerfMode.DoubleRow
        for b in range(NB2):
            if b % 4 == 1 and b // 4 + 1 < 8:
                load_hslices(b // 4 + 1)
            ea_t = eap.tile([JB, 2 * ROWS], FP8, name="ea_t")
            # edge stream rides the scalar-engine HWDGE ring so sync-queue
            # head-of-line stalls on the mask stream cannot delay it
            nc.scalar.dma_start(ea_t[:], eat[b * JB:(b + 1) * JB, :])
            relu_t = var.tile([JB, 2 * ROWS], FP8, name="relu_t")
            nc.scalar.activation(relu_t[:], ea_t[:], AF.Relu)
            min_t = var.tile([JB, 2 * ROWS], FP8, name="min_t")
            nc.vector.tensor_scalar_min(min_t[:], ea_t[:], 0.0)
            pose_t = var.tile([JB, 2 * ROWS], FP8, name="pose_t")
            nc.vector.tensor_single_scalar(pose_t[:], ea_t[:], 0.0, OP.is_gt)

            # both 128-row mask tiles of this block ride one DMA
            pn_t = pnp.tile([JB, 2 * ROWS], BF16, name="pn_t")
            nc.sync.dma_start(pn_t[:], posn[b * JB:(b + 1) * JB, :])
            for u in range(2):
                jb = 2 * b + u
                st = h2ps[:, jb * JB:(jb + 1) * JB]   # [128, 128] bf16 [hi|lo]
                for c in range(NCH):
                    cs = slice(u * ROWS + c * CHUNK, u * ROWS + (c + 1) * CHUNK)
                    nc.tensor.matmul(psP[c][:], st, pn_t[:, cs],
                                     start=(jb == 0), stop=(jb == NJB - 1))

            st8 = h8ps[:, b * 2 * JB:(b + 1) * 2 * JB].rearrange(
                "p (s m) -> p s m", s=2)          # [128, 2, 128] fp8
            on8 = ones8[:].rearrange("p (s m) -> p s m", s=2)  # [128, 2, 16]
            r3 = relu_t[:].rearrange("p (s m) -> p s m", s=2)  # [128, 2, 1024]
            m3 = min_t[:].rearrange("p (s m) -> p s m", s=2)
            g3 = pose_t[:].rearrange("p (s m) -> p s m", s=2)
            sa = (b == 0)
            so = (b == NB2 - 1)
            for c in range(NCH):
                cs = slice(c * CHUNK, (c + 1) * CHUNK)
                nc.tensor.matmul(psR[c][:], st8, r3[:, :, cs],
                                 start=sa, stop=so, perf_mode=DR)
                nc.tensor.matmul(psN[c][:], st8, m3[:, :, cs],
                                 start=sa, stop=so, perf_mode=DR)
                nc.tensor.matmul(psC[c][:], on8, g3[:, :, cs],
                                 start=sa, stop=so, perf_mode=DR)

        # tail-only params: loaded after the stream DMAs are queued so the
        # first adjacency tiles and h2p hit the DMA rings first
        hTp = load_small(hTp_d, [H, ROWS], "hTp")
        hTpr = load_small(hTpr_d, [H, ROWS], "hTpr", F32R)
        WeP = load_small(WeP_d, [2 * H, 3 * H], "WeP", F32R)
        Weh = load_small(Weh_d, [H, 4 * H], "Weh", F32R)
        WnX = load_small(WnX_d, [H, 3 * H], "WnX", F32R)
        Wnh = load_small(Wnh_d, [H, 3 * H], "Wnh", F32R)
        be4 = load_small(be4_d, [H, 4], "be4")
        bn4 = load_small(bn4_d, [H, 4], "bn4")
        Wg2 = load_small(Wg2_d, [2 * H, H], "Wg2", F32R)
        vecsP = load_small(vecsP_d, [2 * H, 2], "vecsP", F32R)
        vech = load_small(vech_d, [H, 2], "vech", F32R)
        cbias = load_small(cbias_d, [1, 2], "cbias")
        ones1 = load_small(ones1_d, [1, H], "ones1", F32R)
        d_er = load_small(d_er_d, [1, ROWS], "d_er", F32R)
        d_nr = load_small(d_nr_d, [1, ROWS], "d_nr", F32R)

        # =================== tail ([64, ROWS] layout, chunk-pipelined) =====
        # copy accumulators to SBUF (P first: scores + edge GRU need it),
        # then release all 8 banks
        cpyP, cpyR, cpyN = [], [], []
        for c in range(NCH):
            tP = work.tile([2 * H, CHUNK], F32R, name=f"cpyP{c}")
            nc.scalar.copy(tP[:], psP[c][:])
            cpyP.append(tP)
        cp_row = work.tile([1, ROWS], F32, name="cp_row")
        for c in range(NCH):
            nc.scalar.copy(cp_row[:, c * CHUNK:(c + 1) * CHUNK], psC[c][0:1, :])
            tR = work.tile([2 * H, CHUNK], F32R, name=f"cpyR{c}")
            nc.scalar.copy(tR[:], psR[c][:])
            cpyR.append(tR)
            tN = work.tile([2 * H, CHUNK], F32R, name=f"cpyN{c}")
            nc.scalar.copy(tN[:], psN[c][:])
            cpyN.append(tN)
        psAcc.release()
        psG = ctx.enter_context(tc.tile_pool(name="psG", bufs=4, space="PSUM"))
        psRow = ctx.enter_context(tc.tile_pool(name="psRow", bufs=2, space="PSUM"))

        # persistent tail tiles (ops run per column chunk for pipelining);
        # row tiles share 7 rotating slots, GRU temps fold into gate tiles
        def wtile(name, shape=None, dt=F32, tag=None, bufs=1):
            return work.tile(shape or [H, ROWS], dt, name=name,
                             **({"tag": tag, "bufs": bufs} if tag else {}))

        def rtile(name):
            return work.tile([1, ROWS], F32, name=name, tag="row", bufs=6)

        ep_pre = rtile("ep_pre"); em_pre = rtile("em_pre")
        ep = rtile("ep"); em = rtile("em")
        m_row = rtile("m_row")
        wp = rtile("wp"); wm = rtile("wm")
        dw = rtile("dw"); tz = rtile("tz")
        z_row = rtile("z_row"); invz = rtile("invz")
        a_p = wtile("a_p", [1, ROWS], F32R); a_m = wtile("a_m", [1, ROWS], F32R)
        spos = wtile("spos"); sneg = wtile("sneg"); es_p = wtile("es_p", dt=F32R)
        ap_b = wtile("ap_b"); am_b = wtile("am_b")
        de_b = wtile("de_b"); dn_b = wtile("dn_b")
        gtiles = {}
        for nm in ("ge", "gn"):
            for t in ("s0", "s1", "s2", "hn"):
                gtiles[f"{nm}_{t}"] = wtile(f"{nm}_{t}")

        def score_mm(c, k, dst):
            cs = slice(c * CHUNK, (c + 1) * CHUNK)
            ps = psRow.tile([1, CHUNK], F32, name="ps_sc", tag="r")
            nc.tensor.matmul(ps[:], vecsP[:, k:k + 1], cpyP[c][:],
                             start=True, stop=False)
            nc.tensor.matmul(ps[:], vech[:, k:k + 1], hTpr[:, cs],
                             start=False, stop=True)
            nc.scalar.activation(dst[:, cs], ps[:], AF.Identity,
                                 bias=cbias[0:1, k:k + 1])

        def mm_copy(dst, c, lhsT, mov, name):
            cs = slice(c * CHUNK, (c + 1) * CHUNK)
            ps = psG.tile([H, CHUNK], F32, name=name, tag="g")
            nc.tensor.matmul(ps[:], lhsT, mov, start=True, stop=True)
            nc.scalar.copy(dst[:, cs], ps[:])

        def gru_chunk(c, nm, xs_P, xs_h, Wh, h_gates, hn_col, bias4):
            """One column-chunk of a GRU; result lands in gtiles[nm_s0]."""
            cs = slice(c * CHUNK, (c + 1) * CHUNK)
            for g, (fn, bcol) in enumerate((("sig", 0), ("sig", 1), ("id", 2))):
                gc = slice(g * H, (g + 1) * H)
                ps = psG.tile([H, CHUNK], F32, name=f"{nm}_g{g}", tag="g")
                mms = [(W_[:, gc], mov[c][:]) for mov, W_ in xs_P]
                mms += [(W_[:, gc], mov[:, cs]) for mov, W_ in xs_h]
                if g < h_gates:
                    mms.append((Wh[:, gc], hTpr[:, cs]))
                for k, (lh, mv) in enumerate(mms):
                    nc.tensor.matmul(ps[:], lh, mv, start=(k == 0),
                                     stop=(k == len(mms) - 1))
                nc.scalar.activation(
                    gtiles[f"{nm}_s{g}"][:, cs], ps[:],
                    AF.Sigmoid if fn == "sig" else AF.Identity,
                    bias=bias4[:, bcol:bcol + 1])
            ps = psG.tile([H, CHUNK], F32, name=f"{nm}_gh", tag="g")
            nc.tensor.matmul(ps[:], Wh[:, hn_col:hn_col + H], hTpr[:, cs],
                             start=True, stop=True)
            nc.scalar.activation(gtiles[f"{nm}_hn"][:, cs], ps[:], AF.Identity,
                                 bias=bias4[:, 3:4])
            r_s, z_s = gtiles[f"{nm}_s0"], gtiles[f"{nm}_s1"]
            ns, hn = gtiles[f"{nm}_s2"], gtiles[f"{nm}_hn"]
            # n = tanh(ns + r*hn); out = n + z*(h - n)   (all in place)
            nc.vector.tensor_tensor(hn[:, cs], r_s[:, cs], hn[:, cs], OP.mult)
            nc.vector.tensor_tensor(ns[:, cs], ns[:, cs], hn[:, cs], OP.add)
            nc.scalar.activation(ns[:, cs], ns[:, cs], AF.Tanh)
            nc.vector.tensor_tensor(r_s[:, cs], hTp[:, cs], ns[:, cs],
                                    OP.subtract)
            nc.vector.tensor_tensor(r_s[:, cs], z_s[:, cs], r_s[:, cs], OP.mult)
            nc.vector.tensor_tensor(r_s[:, cs], r_s[:, cs], ns[:, cs], OP.add)
            return r_s

        for c in range(NCH):
            cs = slice(c * CHUNK, (c + 1) * CHUNK)
            # scores -> leaky relu -> softmax weights -> Z -> a_p/a_m
            score_mm(c, 0, ep_pre)
            score_mm(c, 1, em_pre)
            nc.vector.scalar_tensor_tensor(ep[:, cs], ep_pre[:, cs], ALPHA,
                                           ep_pre[:, cs], OP.mult, OP.max)
            nc.vector.scalar_tensor_tensor(em[:, cs], em_pre[:, cs], ALPHA,
                                           em_pre[:, cs], OP.mult, OP.max)
            nc.vector.tensor_tensor(m_row[:, cs], ep[:, cs], em[:, cs], OP.max)
            nc.vector.tensor_tensor(wp[:, cs], ep[:, cs], m_row[:, cs],
                                    OP.subtract)
            nc.scalar.activation(wp[:, cs], wp[:, cs], AF.Exp)
            nc.vector.tensor_tensor(wm[:, cs], em[:, cs], m_row[:, cs],
                                    OP.subtract)
            nc.scalar.activation(wm[:, cs], wm[:, cs], AF.Exp)
            nc.vector.tensor_tensor(dw[:, cs], wp[:, cs], wm[:, cs],
                                    OP.subtract)
            nc.vector.tensor_tensor(tz[:, cs], dw[:, cs], cp_row[:, cs],
                                    OP.mult)
            nc.vector.scalar_tensor_tensor(z_row[:, cs], wm[:, cs],
                                           float(N - 1), tz[:, cs],
                                           OP.mult, OP.add)
            nc.vector.reciprocal(invz[:, cs], z_row[:, cs])
            nc.vector.tensor_tensor(a_p[:, cs], wp[:, cs], invz[:, cs],
                                    OP.mult)
            nc.vector.tensor_tensor(a_m[:, cs], wm[:, cs], invz[:, cs],
                                    OP.mult)
            # GAT output: es = (ap_b*spos) + (am_b*sneg), in place
            mm_copy(spos, c, Wg2[:], cpyR[c][:], "spos_ps")
            mm_copy(sneg, c, Wg2[:], cpyN[c][:], "sneg_ps")
            mm_copy(ap_b, c, ones1[:], a_p[:, cs], "apb_ps")
            mm_copy(am_b, c, ones1[:], a_m[:, cs], "amb_ps")
            nc.vector.tensor_tensor(spos[:, cs], ap_b[:, cs], spos[:, cs],
                                    OP.mult)
            nc.vector.tensor_tensor(sneg[:, cs], am_b[:, cs], sneg[:, cs],
                                    OP.mult)
            nc.vector.tensor_tensor(es_p[:, cs], spos[:, cs], sneg[:, cs],
                                    OP.add)
            # GRUs
            eo = gru_chunk(c, "ge", [(cpyP, WeP)], [], Weh, 3, 3 * H, be4)
            no = gru_chunk(c, "gn", [], [(es_p, WnX)], Wnh, 2, 2 * H, bn4)
            # final mix (in place into the d bcasts) + store
            mm_copy(de_b, c, ones1[:], d_er[:, cs], "deb_ps")
            mm_copy(dn_b, c, ones1[:], d_nr[:, cs], "dnb_ps")
            nc.vector.tensor_tensor(de_b[:, cs], de_b[:, cs], eo[:, cs],
                                    OP.mult)
            nc.vector.tensor_tensor(dn_b[:, cs], dn_b[:, cs], no[:, cs],
                                    OP.mult)
            nc.vector.tensor_tensor(de_b[:, cs], de_b[:, cs], dn_b[:, cs],
                                    OP.add)
            nc.sync.dma_start(out[:, cs], de_b[:, cs])
        edge_out, node_out = gtiles["ge_s0"], gtiles["gn_s0"]
        fin = de_b

        if DEBUG_DUMP:
            for nm, t in [("d_P", cpyP[0]), ("d_ep", ep), ("d_em", em),
                          ("d_cp", cp_row), ("d_ap", a_p), ("d_am", a_m),
                          ("d_spos", spos), ("d_es", es_p),
                          ("d_eo", edge_out), ("d_no", node_out)]:
                nc.sync.dma_start(dbg[nm][:], t[:].bitcast(mybir.dt.float32))

    _split_multiwaits(nc)
    return nc


def _host_prep(inputs):
    import ml_dtypes

    BF = ml_dtypes.bfloat16
    h = np.ascontiguousarray(inputs["h"], dtype=np.float32)
    node_adj = np.asarray(inputs["node_adj"], dtype=np.float32)
    edge_adj = np.asarray(inputs["edge_adj"], dtype=np.float32)
    W_gat = np.asarray(inputs["W_gat"], dtype=np.float32)
    a_gat = np.asarray(inputs["a_gat"], dtype=np.float32)
    w_ih_e = np.asarray(inputs["w_ih_e"], dtype=np.float32)
    w_hh_e = np.asarray(inputs["w_hh_e"], dtype=np.float32)
    b_ih_e = np.asarray(inputs["b_ih_e"], dtype=np.float32)
    b_hh_e = np.asarray(inputs["b_hh_e"], dtype=np.float32)
    w_ih_n = np.asarray(inputs["w_ih_n"], dtype=np.float32)
    w_hh_n = np.asarray(inputs["w_hh_n"], dtype=np.float32)
    b_ih_n = np.asarray(inputs["b_ih_n"], dtype=np.float32)
    b_hh_n = np.asarray(inputs["b_hh_n"], dtype=np.float32)

    d_node = np.ascontiguousarray(np.diag(node_adj)).astype(np.float32)
    d_edge = np.ascontiguousarray(np.diag(edge_adj)).astype(np.float32)

    FP8 = ml_dtypes.float8_e4m3
    h_hi = h.astype(BF).astype(np.float32)
    h_lo = (h - h_hi).astype(BF).astype(np.float32)
    h8_hi = h.astype(FP8).astype(np.float32)
    h8_lo = (h - h8_hi).astype(FP8)
    sum_h = h.sum(axis=0, dtype=np.float64).astype(np.float32)    # [H]

    # h2p [128, N]: h2p[p, jb*128+m] = (m<64 ? h_hi : h_lo)[jb*128+p, m%64]
    hi3 = h_hi.reshape(NJB, JB, H).transpose(1, 0, 2)
    lo3 = h_lo.reshape(NJB, JB, H).transpose(1, 0, 2)
    h2p = np.concatenate([hi3, lo3], axis=2).reshape(JB, N).astype(BF)
    # h8p [128, N] fp8 for DoubleRow: (p, b*256 + s*128 + m) = h8[b*256+2p+s, m]
    h8cat = np.concatenate([h8_hi.astype(FP8).astype(np.float32),
                            h8_lo.astype(np.float32)], axis=1)      # [N, 128]
    h8p = (h8cat.reshape(NB2, JB, 2, 2 * H).transpose(1, 0, 2, 3)
           .reshape(JB, N)).astype(FP8)

    a1 = a_gat[0:H, 0]
    a2 = a_gat[H:2 * H, 0]
    Wa1 = W_gat @ a1
    Wa2 = W_gat @ a2

    def stack2(x):
        return np.ascontiguousarray(np.concatenate([x, x], axis=0),
                                    dtype=np.float32)

    vecsP = stack2(np.stack([Wa1 - Wa2, Wa2 - Wa1], axis=1))
    vech = np.ascontiguousarray(np.stack([-Wa2, -Wa1], axis=1), np.float32)
    cbias = np.array([[float(sum_h @ Wa2), float(sum_h @ Wa1)]], np.float32)

    wieP = np.ascontiguousarray(w_ih_e.T[0:H, :])       # [64, 192]
    wieM = np.ascontiguousarray(w_ih_e.T[H:2 * H, :])
    whhe = np.ascontiguousarray(w_hh_e.T)               # [64, 192]
    wihn = np.ascontiguousarray(w_ih_n.T)
    whhn = np.ascontiguousarray(w_hh_n.T)

    WeP = stack2(wieP - wieM)
    Weh = np.zeros((H, 4 * H), np.float32)
    Weh[:, 0:2 * H] = -wieM[:, 0:2 * H] + whhe[:, 0:2 * H]        # r|z
    Weh[:, 2 * H:3 * H] = -wieM[:, 2 * H:3 * H]                   # in
    Weh[:, 3 * H:4 * H] = whhe[:, 2 * H:3 * H]                    # hn
    WnX = np.ascontiguousarray(wihn)
    Wnh = np.zeros((H, 3 * H), np.float32)
    Wnh[:, 0:2 * H] = whhn[:, 0:2 * H]                            # r|z
    Wnh[:, 2 * H:3 * H] = whhn[:, 2 * H:3 * H]                    # hn

    be4 = np.zeros((H, 4), np.float32)
    be4[:, 0] = b_ih_e[0:H] + b_hh_e[0:H] + wieM[:, 0:H].T @ sum_h
    be4[:, 1] = (b_ih_e[H:2 * H] + b_hh_e[H:2 * H]
                 + wieM[:, H:2 * H].T @ sum_h)
    be4[:, 2] = b_ih_e[2 * H:3 * H] + wieM[:, 2 * H:3 * H].T @ sum_h
    be4[:, 3] = b_hh_e[2 * H:3 * H]
    bn4 = np.zeros((H, 4), np.float32)
    bn4[:, 0] = b_ih_n[0:H] + b_hh_n[0:H]
    bn4[:, 1] = b_ih_n[H:2 * H] + b_hh_n[H:2 * H]
    bn4[:, 2] = b_ih_n[2 * H:3 * H]
    bn4[:, 3] = b_hh_n[2 * H:3 * H]

    shared = {
        "h2p": h2p, "h8p": h8p,
        "ones8": np.ones((JB, 32), FP8),
        "WeP": WeP, "Weh": Weh, "WnX": WnX, "Wnh": Wnh,
        "be4": be4, "bn4": bn4,
        "Wg2": stack2(W_gat), "vecsP": vecsP, "vech": vech, "cbias": cbias,
        "ones1": np.ones((1, H), np.float32),
    }

    nat_full = np.ascontiguousarray(node_adj.T)
    eat_full = np.ascontiguousarray(edge_adj.T)
    idx = np.arange(ROWS)
    in_maps = []
    for c in range(NCORES):
        sl = slice(c * ROWS, (c + 1) * ROWS)
        nat = nat_full[:, sl].copy()
        nat[c * ROWS + idx, idx] = 0.0
        eat = eat_full[:, sl].copy()
        eat[c * ROWS + idx, idx] = 0.0
        m = dict(shared)
        pn_mask = (nat > 0).astype(np.float32)
        m["posn"] = np.ascontiguousarray(
            pn_mask.reshape(NB2, 2, JB, ROWS).transpose(0, 2, 1, 3)
            .reshape(N // 2, 2 * ROWS)).astype(BF)
        m["eat"] = np.ascontiguousarray(
            eat.astype(FP8).reshape(N // 2, 2 * ROWS))
        hTp = np.ascontiguousarray(h[sl].T)
        m["hTp"] = hTp
        m["hTpr"] = hTp
        m["d_er"] = d_edge[sl].reshape(1, ROWS).copy()
        m["d_nr"] = d_node[sl].reshape(1, ROWS).copy()
        in_maps.append(m)
    return in_maps


def _unshard(outs):
    full = np.empty((N, H), np.float32)
    for c in range(NCORES):
        full[c * ROWS:(c + 1) * ROWS, :] = outs[c].T   # [64, 1024] -> rows
    return full


def _run(inputs, trace=False, tmpdir=None):
    from concourse.bass_utils import run_bass_kernel_spmd

    in_maps = _host_prep(inputs)
    nc = _build_nc()
    res = run_bass_kernel_spmd(nc, in_maps, core_ids=list(range(NCORES)),
                               trace=trace, tmpdir=tmpdir)
    full = _unshard([res.results[c]["out"] for c in range(NCORES)])
    return np.ascontiguousarray(full, dtype=np.float32), res


def kernel(**inputs):
    out, _ = _run(inputs, trace=False)
    return out


# revision 31
# speedup vs baseline: 1.1461x; 1.1461x over previous
"""Trainium2 Bass kernel for nn_FactorGraphGRU (N=8192, H=64, 8 NeuronCores).

Strategy (memory-bound): row-shard the outputs across 8 cores (1024 each).
Each core streams transposed adjacency shards once from HBM in bf16:

  posn  [N, 1024] bf16  host-built positive mask of node_adj^T (exact 0/1)
  eat   [N, 1024] bf16  edge_adj^T values (bf16 round ~0.4%, tolerance 2e-2)

Per 128-row block the tensor engine runs 4 matmul passes against a
stationary [h_hi | h_lo] bf16 tile (hi/lo split keeps the attention-score
exponents accurate): P (node mask), R=relu(eat), Nm=min(eat,0), count
(pos_e vs ones).  relu on ACT, min/is_gt on DVE (bf16 fast modes); the
GPSIMD engine is never used (its elementwise path measured ~20x slower).

All downstream algebra is folded into host-precomputed stationaries:
  - M = sum_h - h_i - P is eliminated (coefficients on P/h + bias consts)
  - hi/lo recombine is folded into every consumer stationary ([W; W])
The tail runs in the [64, ROWS] transposed layout (this toolchain cannot
encode matmul outputs at a non-zero PSUM base partition).  The GAT softmax
collapses to the two-value form: es = a_p*(W^T R) + a_m*(W^T Nm), with
Z = cp*(wp-wm) + (N-1)*wm from the streamed positive-count row.
"""

import numpy as np
from contextlib import ExitStack

N = 8192
H = 64
NCORES = 8
ROWS = N // NCORES        # 1024 output rows per core
JB = 128                  # contraction block (SBUF partitions)
NJB = N // JB             # 64
NB2 = N // (2 * JB)       # 32 fp8 DoubleRow blocks (256 rows each)
CHUNK = 512               # PSUM bank free size (f32)
NCH = ROWS // CHUNK       # 2
ALPHA = 0.2               # leaky relu slope
DEBUG_DUMP = False        # test hook: dump intermediates as extra outputs
USE_FAST_RECIP = True     # custom-DVE reciprocal (falls back to stock op)


# ---------------------------------------------------------------------------
# walrus workaround: this toolchain accepts at most ONE sync wait per
# instruction; Tile attaches several.  Rewrite the BIR so every extra wait
# rides on its own NoOp carrier right before the instruction.
# ---------------------------------------------------------------------------
def _split_multiwaits(nc):
    import bass_rust
    import concourse.mybir as mybir

    ctr = [0]

    def carrier(engine, wait):
        ctr[0] += 1
        nop = bass_rust.InstNoOp(name=f"WS-{ctr[0]}", engine=engine, ins=[], outs=[])
        nop.sync_info = mybir.SyncInfo(on_wait=[wait], on_update=[])
        return nop

    for fn in nc.m.functions:
        stack = list(fn.blocks)
        while stack:
            bb = stack.pop()
            stack.extend(getattr(bb, "blocks", []) or [])
            out = []
            changed = False
            for inst in bb.instructions:
                si = inst.sync_info
                waits = list(si.on_wait) if si is not None and si.on_wait else []
                if len(waits) > 1:
                    for w in waits[:-1]:
                        out.append(carrier(inst.engine, w))
                    si.on_wait = [waits[-1]]
                    changed = True
                out.append(inst)
            if changed:
                bb.instructions = out
    return nc


def _build_nc():
    import concourse.bass as bass
    import concourse.tile as tile
    from concourse import mybir

    F32 = mybir.dt.float32
    F32R = mybir.dt.float32r
    BF16 = mybir.dt.bfloat16
    FP8 = mybir.dt.float8e4
    AF = mybir.ActivationFunctionType
    OP = mybir.AluOpType

    nc = bass.Bass("TRN2", target_bir_lowering=False, debug=False,
                   num_devices=NCORES)

    # --- DRAM inputs (per-core shards via in_maps) ---
    comb = nc.dram_tensor("comb", [N // 2, 6 * ROWS], FP8, kind="ExternalInput").ap()
    h2p_d = nc.dram_tensor("h2p", [JB, N], BF16, kind="ExternalInput").ap()
    h8p_d = nc.dram_tensor("h8p", [JB, N], FP8, kind="ExternalInput").ap()
    ones8_d = nc.dram_tensor("ones8", [JB, 32], FP8, kind="ExternalInput").ap()
    hTp_d = nc.dram_tensor("hTp", [H, ROWS], F32, kind="ExternalInput").ap()
    hTpr_d = nc.dram_tensor("hTpr", [H, ROWS], F32R, kind="ExternalInput").ap()
    WeP_d = nc.dram_tensor("WeP", [2 * H, 3 * H], F32R, kind="ExternalInput").ap()
    Weh_d = nc.dram_tensor("Weh", [H, 4 * H], F32R, kind="ExternalInput").ap()
    WnX_d = nc.dram_tensor("WnX", [H, 3 * H], F32R, kind="ExternalInput").ap()
    Wnh_d = nc.dram_tensor("Wnh", [H, 3 * H], F32R, kind="ExternalInput").ap()
    be4_d = nc.dram_tensor("be4", [H, 4], F32, kind="ExternalInput").ap()
    bn4_d = nc.dram_tensor("bn4", [H, 4], F32, kind="ExternalInput").ap()
    Wg2_d = nc.dram_tensor("Wg2", [2 * H, H], F32R, kind="ExternalInput").ap()
    vecsP_d = nc.dram_tensor("vecsP", [2 * H, 2], F32R, kind="ExternalInput").ap()
    vech_d = nc.dram_tensor("vech", [H, 2], F32R, kind="ExternalInput").ap()
    cbias_d = nc.dram_tensor("cbias", [1, 2], F32, kind="ExternalInput").ap()
    ones1_d = nc.dram_tensor("ones1", [1, H], F32R, kind="ExternalInput").ap()
    d_er_d = nc.dram_tensor("d_er", [1, ROWS], F32R, kind="ExternalInput").ap()
    d_nr_d = nc.dram_tensor("d_nr", [1, ROWS], F32R, kind="ExternalInput").ap()
    out = nc.dram_tensor("out", [H, ROWS], F32, kind="ExternalOutput").ap()
    dbg = {}
    if DEBUG_DUMP:
        for nm, sh in [("d_P", [2 * H, CHUNK]), ("d_ep", [1, ROWS]),
                       ("d_em", [1, ROWS]), ("d_cp", [1, ROWS]),
                       ("d_ap", [1, ROWS]), ("d_am", [1, ROWS]),
                       ("d_spos", [H, ROWS]), ("d_es", [H, ROWS]),
                       ("d_eo", [H, ROWS]), ("d_no", [H, ROWS])]:
            dbg[nm] = nc.dram_tensor(nm, sh, F32, kind="ExternalOutput").ap()

    with tile.TileContext(nc) as tc, ExitStack() as ctx:
        # --- pools ---
        cbp = ctx.enter_context(tc.tile_pool(name="cbp", bufs=4))
        var = ctx.enter_context(tc.tile_pool(name="var", bufs=4))
        small = ctx.enter_context(tc.tile_pool(name="small", bufs=1))
        work = ctx.enter_context(tc.tile_pool(name="work", bufs=1))
        psAcc = tc.alloc_tile_pool(name="psAcc", bufs=1, space="PSUM")

        # --- small persistent inputs ---
        def load_small(src, shape, name, dt=F32):
            t = small.tile(shape, dt, name=name)
            nc.sync.dma_start(t[:], src[:])
            return t

        # h2p/h8p loaded in 8 slices just-in-time (slice q covers blocks
        # [4q, 4q+4); q+1 is issued at block 4q+1, three blocks of lead)
        h2ps = small.tile([JB, N], BF16, name="h2ps")
        h8ps = small.tile([JB, N], FP8, name="h8ps")

        def load_hslices(q):
            qs = slice(q * (N // 8), (q + 1) * (N // 8))
            nc.sync.dma_start(h2ps[:, qs], h2p_d[:, qs])
            nc.sync.dma_start(h8ps[:, qs], h8p_d[:, qs])

        load_hslices(0)
        ones8 = load_small(ones8_d, [JB, 32], "ones8", FP8)

        # --- PSUM accumulators: 6x[128,512] + 2x[1,512] = 8 banks ---
        psP = [psAcc.tile([2 * H, CHUNK], F32, name=f"psP{c}", tag=f"psP{c}")
               for c in range(NCH)]
        psR = [psAcc.tile([2 * H, CHUNK], F32, name=f"psR{c}", tag=f"psR{c}")
               for c in range(NCH)]
        psN = [psAcc.tile([2 * H, CHUNK], F32, name=f"psN{c}", tag=f"psN{c}")
               for c in range(NCH)]
        psC = [psAcc.tile([16, CHUNK], F32, name=f"psC{c}", tag=f"psC{c}")
               for c in range(NCH)]

        # =================== stream: one pass over both adjacencies ========
        # node mask: bf16 128-row tiles; edge: fp8 DoubleRow 256-row blocks
        DR = mybir.MatmulPerfMode.DoubleRow
        for b in range(NB2):
            if b % 4 == 1 and b // 4 + 1 < 8:
                load_hslices(b // 4 + 1)
            # mask+edge bytes ride one DMA: [0:4096)=bf16 mask, [4096:6144)=fp8 edge
            cb_t = cbp.tile([JB, 6 * ROWS], FP8, name="cb_t")
            nc.sync.dma_start(cb_t[:], comb[b * JB:(b + 1) * JB, :])
            ea_t = cb_t[:, 4 * ROWS:6 * ROWS]
            relu_t = var.tile([JB, 2 * ROWS], FP8, name="relu_t")
            nc.scalar.activation(relu_t[:], ea_t[:], AF.Relu)
            min_t = var.tile([JB, 2 * ROWS], FP8, name="min_t")
            nc.vector.tensor_scalar_min(min_t[:], ea_t[:], 0.0)
            pose_t = var.tile([JB, 2 * ROWS], FP8, name="pose_t")
            nc.vector.tensor_single_scalar(pose_t[:], ea_t[:], 0.0, OP.is_gt)

            pn_t = cb_t[:].bitcast(BF16)[:, 0:2 * ROWS]   # bf16 mask view
            for u in range(2):
                jb = 2 * b + u
                st = h2ps[:, jb * JB:(jb + 1) * JB]   # [128, 128] bf16 [hi|lo]
                for c in range(NCH):
                    cs = slice(u * ROWS + c * CHUNK, u * ROWS + (c + 1) * CHUNK)
                    nc.tensor.matmul(psP[c][:], st, pn_t[:, cs],
                                     start=(jb == 0), stop=(jb == NJB - 1))

            st8 = h8ps[:, b * 2 * JB:(b + 1) * 2 * JB].rearrange(
                "p (s m) -> p s m", s=2)          # [128, 2, 128] fp8
            on8 = ones8[:].rearrange("p (s m) -> p s m", s=2)  # [128, 2, 16]
            r3 = relu_t[:].rearrange("p (s m) -> p s m", s=2)  # [128, 2, 1024]
            m3 = min_t[:].rearrange("p (s m) -> p s m", s=2)
            g3 = pose_t[:].rearrange("p (s m) -> p s m", s=2)
            sa = (b == 0)
            so = (b == NB2 - 1)
            for c in range(NCH):
                cs = slice(c * CHUNK, (c + 1) * CHUNK)
                nc.tensor.matmul(psR[c][:], st8, r3[:, :, cs],
                                 start=sa, stop=so, perf_mode=DR)
                nc.tensor.matmul(psN[c][:], st8, m3[:, :, cs],
                                 start=sa, stop=so, perf_mode=DR)
                nc.tensor.matmul(psC[c][:], on8, g3[:, :, cs],
                                 start=sa, stop=so, perf_mode=DR)

        # tail-only params: loaded after the stream DMAs are queued so the
        # first adjacency tiles and h2p hit the DMA rings first
        hTp = load_small(hTp_d, [H, ROWS], "hTp")
        hTpr = load_small(hTpr_d, [H, ROWS], "hTpr", F32R)
        WeP = load_small(WeP_d, [2 * H, 3 * H], "WeP", F32R)
        Weh = load_small(Weh_d, [H, 4 * H], "Weh", F32R)
        WnX = load_small(WnX_d, [H, 3 * H], "WnX", F32R)
        Wnh = load_small(Wnh_d, [H, 3 * H], "Wnh", F32R)
        be4 = load_small(be4_d, [H, 4], "be4")
        bn4 = load_small(bn4_d, [H, 4], "bn4")
        Wg2 = load_small(Wg2_d, [2 * H, H], "Wg2", F32R)
        vecsP = load_small(vecsP_d, [2 * H, 2], "vecsP", F32R)
        vech = load_small(vech_d, [H, 2], "vech", F32R)
        cbias = load_small(cbias_d, [1, 2], "cbias")
        ones1 = load_small(ones1_d, [1, H], "ones1", F32R)
        d_er = load_small(d_er_d, [1, ROWS], "d_er", F32R)
        d_nr = load_small(d_nr_d, [1, ROWS], "d_nr", F32R)

        # =================== tail ([64, ROWS] layout, chunk-pipelined) =====
        # copy accumulators to SBUF (P first: scores + edge GRU need it),
        # then release all 8 banks
        cpyP, cpyR, cpyN = [], [], []
        for c in range(NCH):
            tP = work.tile([2 * H, CHUNK], F32R, name=f"cpyP{c}")
            nc.scalar.copy(tP[:], psP[c][:])
            cpyP.append(tP)
        cp_row = work.tile([1, ROWS], F32, name="cp_row")
        for c in range(NCH):
            nc.scalar.copy(cp_row[:, c * CHUNK:(c + 1) * CHUNK], psC[c][0:1, :])
            tR = work.tile([2 * H, CHUNK], F32R, name=f"cpyR{c}")
            nc.scalar.copy(tR[:], psR[c][:])
            cpyR.append(tR)
            tN = work.tile([2 * H, CHUNK], F32R, name=f"cpyN{c}")
            nc.scalar.copy(tN[:], psN[c][:])
            cpyN.append(tN)
        psAcc.release()
        psG = ctx.enter_context(tc.tile_pool(name="psG", bufs=4, space="PSUM"))
        psRow = ctx.enter_context(tc.tile_pool(name="psRow", bufs=2, space="PSUM"))

        # persistent tail tiles (ops run per column chunk for pipelining);
        # row tiles share 7 rotating slots, GRU temps fold into gate tiles
        def wtile(name, shape=None, dt=F32, tag=None, bufs=1):
            return work.tile(shape or [H, ROWS], dt, name=name,
                             **({"tag": tag, "bufs": bufs} if tag else {}))

        def rtile(name):
            return work.tile([1, ROWS], F32, name=name, tag="row", bufs=6)

        ep_pre = rtile("ep_pre"); em_pre = rtile("em_pre")
        ep = rtile("ep"); em = rtile("em")
        m_row = rtile("m_row")
        wp = rtile("wp"); wm = rtile("wm")
        dw = rtile("dw"); tz = rtile("tz")
        z_row = rtile("z_row"); invz = rtile("invz")
        a_p = wtile("a_p", [1, ROWS], F32R); a_m = wtile("a_m", [1, ROWS], F32R)
        spos = wtile("spos"); sneg = wtile("sneg"); es_p = wtile("es_p", dt=F32R)
        ap_b = wtile("ap_b"); am_b = wtile("am_b")
        de_b = wtile("de_b"); dn_b = wtile("dn_b")
        gtiles = {}
        for nm in ("ge", "gn"):
            for t in ("s0", "s1", "s2", "hn"):
                gtiles[f"{nm}_{t}"] = wtile(f"{nm}_{t}")

        def score_mm(c, k, dst):
            cs = slice(c * CHUNK, (c + 1) * CHUNK)
            ps = psRow.tile([1, CHUNK], F32, name="ps_sc", tag="r")
            nc.tensor.matmul(ps[:], vecsP[:, k:k + 1], cpyP[c][:],
                             start=True, stop=False)
            nc.tensor.matmul(ps[:], vech[:, k:k + 1], hTpr[:, cs],
                             start=False, stop=True)
            nc.scalar.activation(dst[:, cs], ps[:], AF.Identity,
                                 bias=cbias[0:1, k:k + 1])

        def mm_copy(dst, c, lhsT, mov, name):
            cs = slice(c * CHUNK, (c + 1) * CHUNK)
            ps = psG.tile([H, CHUNK], F32, name=name, tag="g")
            nc.tensor.matmul(ps[:], lhsT, mov, start=True, stop=True)
            nc.scalar.copy(dst[:, cs], ps[:])

        def gru_chunk(c, nm, xs_P, xs_h, Wh, h_gates, hn_col, bias4):
            """One column-chunk of a GRU; result lands in gtiles[nm_s0]."""
            cs = slice(c * CHUNK, (c + 1) * CHUNK)
            for g, (fn, bcol) in enumerate((("sig", 0), ("sig", 1), ("id", 2))):
                gc = slice(g * H, (g + 1) * H)
                ps = psG.tile([H, CHUNK], F32, name=f"{nm}_g{g}", tag="g")
                mms = [(W_[:, gc], mov[c][:]) for mov, W_ in xs_P]
                mms += [(W_[:, gc], mov[:, cs]) for mov, W_ in xs_h]
                if g < h_gates:
                    mms.append((Wh[:, gc], hTpr[:, cs]))
                for k, (lh, mv) in enumerate(mms):
                    nc.tensor.matmul(ps[:], lh, mv, start=(k == 0),
                                     stop=(k == len(mms) - 1))
                nc.scalar.activation(
                    gtiles[f"{nm}_s{g}"][:, cs], ps[:],
                    AF.Sigmoid if fn == "sig" else AF.Identity,
                    bias=bias4[:, bcol:bcol + 1])
            ps = psG.tile([H, CHUNK], F32, name=f"{nm}_gh", tag="g")
            nc.tensor.matmul(ps[:], Wh[:, hn_col:hn_col + H], hTpr[:, cs],
                             start=True, stop=True)
            nc.scalar.activation(gtiles[f"{nm}_hn"][:, cs], ps[:], AF.Identity,
                                 bias=bias4[:, 3:4])
            r_s, z_s = gtiles[f"{nm}_s0"], gtiles[f"{nm}_s1"]
            ns, hn = gtiles[f"{nm}_s2"], gtiles[f"{nm}_hn"]
            # n = tanh(ns + r*hn); out = n + z*(h - n)   (all in place)
            nc.vector.tensor_tensor(hn[:, cs], r_s[:, cs], hn[:, cs], OP.mult)
            nc.vector.tensor_tensor(ns[:, cs], ns[:, cs], hn[:, cs], OP.add)
            nc.scalar.activation(ns[:, cs], ns[:, cs], AF.Tanh)
            nc.vector.tensor_tensor(r_s[:, cs], hTp[:, cs], ns[:, cs],
                                    OP.subtract)
            nc.vector.tensor_tensor(r_s[:, cs], z_s[:, cs], r_s[:, cs], OP.mult)
            nc.vector.tensor_tensor(r_s[:, cs], r_s[:, cs], ns[:, cs], OP.add)
            return r_s

        for c in range(NCH):
            cs = slice(c * CHUNK, (c + 1) * CHUNK)
            # scores -> leaky relu -> softmax weights -> Z -> a_p/a_m
            score_mm(c, 0, ep_pre)
            score_mm(c, 1, em_pre)
            nc.vector.scalar_tensor_tensor(ep[:, cs], ep_pre[:, cs], ALPHA,
                                           ep_pre[:, cs], OP.mult, OP.max)
            nc.vector.scalar_tensor_tensor(em[:, cs], em_pre[:, cs], ALPHA,
                                           em_pre[:, cs], OP.mult, OP.max)
            nc.vector.tensor_tensor(m_row[:, cs], ep[:, cs], em[:, cs], OP.max)
            nc.vector.tensor_tensor(wp[:, cs], ep[:, cs], m_row[:, cs],
                                    OP.subtract)
            nc.scalar.activation(wp[:, cs], wp[:, cs], AF.Exp)
            nc.vector.tensor_tensor(wm[:, cs], em[:, cs], m_row[:, cs],
                                    OP.subtract)
            nc.scalar.activation(wm[:, cs], wm[:, cs], AF.Exp)
            nc.vector.tensor_tensor(dw[:, cs], wp[:, cs], wm[:, cs],
                                    OP.subtract)
            nc.vector.tensor_tensor(tz[:, cs], dw[:, cs], cp_row[:, cs],
                                    OP.mult)
            nc.vector.scalar_tensor_tensor(z_row[:, cs], wm[:, cs],
                                           float(N - 1), tz[:, cs],
                                           OP.mult, OP.add)
            nc.vector.reciprocal(invz[:, cs], z_row[:, cs])
            nc.vector.tensor_tensor(a_p[:, cs], wp[:, cs], invz[:, cs],
                                    OP.mult)
            nc.vector.tensor_tensor(a_m[:, cs], wm[:, cs], invz[:, cs],
                                    OP.mult)
            # GAT output: es = (ap_b*spos) + (am_b*sneg), in place
            mm_copy(spos, c, Wg2[:], cpyR[c][:], "spos_ps")
            mm_copy(sneg, c, Wg2[:], cpyN[c][:], "sneg_ps")
            mm_copy(ap_b, c, ones1[:], a_p[:, cs], "apb_ps")
            mm_copy(am_b, c, ones1[:], a_m[:, cs], "amb_ps")
            nc.vector.tensor_tensor(spos[:, cs], ap_b[:, cs], spos[:, cs],
                                    OP.mult)
            nc.vector.tensor_tensor(sneg[:, cs], am_b[:, cs], sneg[:, cs],
                                    OP.mult)
            nc.vector.tensor_tensor(es_p[:, cs], spos[:, cs], sneg[:, cs],
                                    OP.add)
            # GRUs
            eo = gru_chunk(c, "ge", [(cpyP, WeP)], [], Weh, 3, 3 * H, be4)
            no = gru_chunk(c, "gn", [], [(es_p, WnX)], Wnh, 2, 2 * H, bn4)
            # final mix (in place into the d bcasts) + store
            mm_copy(de_b, c, ones1[:], d_er[:, cs], "deb_ps")
            mm_copy(dn_b, c, ones1[:], d_nr[:, cs], "dnb_ps")
            nc.vector.tensor_tensor(de_b[:, cs], de_b[:, cs], eo[:, cs],
                                    OP.mult)
            nc.vector.tensor_tensor(dn_b[:, cs], dn_b[:, cs], no[:, cs],
                                    OP.mult)
            nc.vector.tensor_tensor(de_b[:, cs], de_b[:, cs], dn_b[:, cs],
                                    OP.add)
            nc.sync.dma_start(out[:, cs], de_b[:, cs])
        edge_out, node_out = gtiles["ge_s0"], gtiles["gn_s0"]
        fin = de_b

        if DEBUG_DUMP:
            for nm, t in [("d_P", cpyP[0]), ("d_ep", ep), ("d_em", em),
                          ("d_cp", cp_row), ("d_ap", a_p), ("d_am", a_m),
                          ("d_spos", spos), ("d_es", es_p),
                          ("d_eo", edge_out), ("d_no", node_out)]:
                nc.sync.dma_start(dbg[nm][:], t[:].bitcast(mybir.dt.float32))

    _split_multiwaits(nc)
    return nc


def _host_prep(inputs):
    import ml_dtypes

    BF = ml_dtypes.bfloat16
    h = np.ascontiguousarray(inputs["h"], dtype=np.float32)
    node_adj = np.asarray(inputs["node_adj"], dtype=np.float32)
    edge_adj = np.asarray(inputs["edge_adj"], dtype=np.float32)
    W_gat = np.asarray(inputs["W_gat"], dtype=np.float32)
    a_gat = np.asarray(inputs["a_gat"], dtype=np.float32)
    w_ih_e = np.asarray(inputs["w_ih_e"], dtype=np.float32)
    w_hh_e = np.asarray(inputs["w_hh_e"], dtype=np.float32)
    b_ih_e = np.asarray(inputs["b_ih_e"], dtype=np.float32)
    b_hh_e = np.asarray(inputs["b_hh_e"], dtype=np.float32)
    w_ih_n = np.asarray(inputs["w_ih_n"], dtype=np.float32)
    w_hh_n = np.asarray(inputs["w_hh_n"], dtype=np.float32)
    b_ih_n = np.asarray(inputs["b_ih_n"], dtype=np.float32)
    b_hh_n = np.asarray(inputs["b_hh_n"], dtype=np.float32)

    d_node = np.ascontiguousarray(np.diag(node_adj)).astype(np.float32)
    d_edge = np.ascontiguousarray(np.diag(edge_adj)).astype(np.float32)

    FP8 = ml_dtypes.float8_e4m3
    h_hi = h.astype(BF).astype(np.float32)
    h_lo = (h - h_hi).astype(BF).astype(np.float32)
    h8_hi = h.astype(FP8).astype(np.float32)
    h8_lo = (h - h8_hi).astype(FP8)
    sum_h = h.sum(axis=0, dtype=np.float64).astype(np.float32)    # [H]

    # h2p [128, N]: h2p[p, jb*128+m] = (m<64 ? h_hi : h_lo)[jb*128+p, m%64]
    hi3 = h_hi.reshape(NJB, JB, H).transpose(1, 0, 2)
    lo3 = h_lo.reshape(NJB, JB, H).transpose(1, 0, 2)
    h2p = np.concatenate([hi3, lo3], axis=2).reshape(JB, N).astype(BF)
    # h8p [128, N] fp8 for DoubleRow: (p, b*256 + s*128 + m) = h8[b*256+2p+s, m]
    h8cat = np.concatenate([h8_hi.astype(FP8).astype(np.float32),
                            h8_lo.astype(np.float32)], axis=1)      # [N, 128]
    h8p = (h8cat.reshape(NB2, JB, 2, 2 * H).transpose(1, 0, 2, 3)
           .reshape(JB, N)).astype(FP8)

    a1 = a_gat[0:H, 0]
    a2 = a_gat[H:2 * H, 0]
    Wa1 = W_gat @ a1
    Wa2 = W_gat @ a2

    def stack2(x):
        return np.ascontiguousarray(np.concatenate([x, x], axis=0),
                                    dtype=np.float32)

    vecsP = stack2(np.stack([Wa1 - Wa2, Wa2 - Wa1], axis=1))
    vech = np.ascontiguousarray(np.stack([-Wa2, -Wa1], axis=1), np.float32)
    cbias = np.array([[float(sum_h @ Wa2), float(sum_h @ Wa1)]], np.float32)

    wieP = np.ascontiguousarray(w_ih_e.T[0:H, :])       # [64, 192]
    wieM = np.ascontiguousarray(w_ih_e.T[H:2 * H, :])
    whhe = np.ascontiguousarray(w_hh_e.T)               # [64, 192]
    wihn = np.ascontiguousarray(w_ih_n.T)
    whhn = np.ascontiguousarray(w_hh_n.T)

    WeP = stack2(wieP - wieM)
    Weh = np.zeros((H, 4 * H), np.float32)
    Weh[:, 0:2 * H] = -wieM[:, 0:2 * H] + whhe[:, 0:2 * H]        # r|z
    Weh[:, 2 * H:3 * H] = -wieM[:, 2 * H:3 * H]                   # in
    Weh[:, 3 * H:4 * H] = whhe[:, 2 * H:3 * H]                    # hn
    WnX = np.ascontiguousarray(wihn)
    Wnh = np.zeros((H, 3 * H), np.float32)
    Wnh[:, 0:2 * H] = whhn[:, 0:2 * H]                            # r|z
    Wnh[:, 2 * H:3 * H] = whhn[:, 2 * H:3 * H]                    # hn

    be4 = np.zeros((H, 4), np.float32)
    be4[:, 0] = b_ih_e[0:H] + b_hh_e[0:H] + wieM[:, 0:H].T @ sum_h
    be4[:, 1] = (b_ih_e[H:2 * H] + b_hh_e[H:2 * H]
                 + wieM[:, H:2 * H].T @ sum_h)
    be4[:, 2] = b_ih_e[2 * H:3 * H] + wieM[:, 2 * H:3 * H].T @ sum_h
    be4[:, 3] = b_hh_e[2 * H:3 * H]
    bn4 = np.zeros((H, 4), np.float32)
    bn4[:, 0] = b_ih_n[0:H] + b_hh_n[0:H]
    bn4[:, 1] = b_ih_n[H:2 * H] + b_hh_n[H:2 * H]
    bn4[:, 2] = b_ih_n[2 * H:3 * H]
    bn4[:, 3] = b_hh_n[2 * H:3 * H]

    shared = {
        "h2p": h2p, "h8p": h8p,
        "ones8": np.ones((JB, 32), FP8),
        "WeP": WeP, "Weh": Weh, "WnX": WnX, "Wnh": Wnh,
        "be4": be4, "bn4": bn4,
        "Wg2": stack2(W_gat), "vecsP": vecsP, "vech": vech, "cbias": cbias,
        "ones1": np.ones((1, H), np.float32),
    }

    nat_full = np.ascontiguousarray(node_adj.T)
    eat_full = np.ascontiguousarray(edge_adj.T)
    idx = np.arange(ROWS)
    in_maps = []
    for c in range(NCORES):
        sl = slice(c * ROWS, (c + 1) * ROWS)
        nat = nat_full[:, sl].copy()
        nat[c * ROWS + idx, idx] = 0.0
        eat = eat_full[:, sl].copy()
        eat[c * ROWS + idx, idx] = 0.0
        m = dict(shared)
        pn_mask = (nat > 0).astype(np.float32)
        pn_b = np.ascontiguousarray(
            pn_mask.reshape(NB2, 2, JB, ROWS).transpose(0, 2, 1, 3)
            .reshape(N // 2, 2 * ROWS)).astype(BF)
        ea_b = np.ascontiguousarray(eat.astype(FP8).reshape(N // 2, 2 * ROWS))
        m["comb"] = np.ascontiguousarray(np.concatenate(
            [pn_b.view(np.uint8), ea_b.view(np.uint8)], axis=1)).view(FP8)
        hTp = np.ascontiguousarray(h[sl].T)
        m["hTp"] = hTp
        m["hTpr"] = hTp
        m["d_er"] = d_edge[sl].reshape(1, ROWS).copy()
        m["d_nr"] = d_node[sl].reshape(1, ROWS).copy()
        in_maps.append(m)
    return in_maps


def _unshard(outs):
    full = np.empty((N, H), np.float32)
    for c in range(NCORES):
        full[c * ROWS:(c + 1) * ROWS, :] = outs[c].T   # [64, 1024] -> rows
    return full


def _run(inputs, trace=False, tmpdir=None):
    from concourse.bass_utils import run_bass_kernel_spmd

    in_maps = _host_prep(inputs)
    nc = _build_nc()
    res = run_bass_kernel_spmd(nc, in_maps, core_ids=list(range(NCORES)),
                               trace=trace, tmpdir=tmpdir)
    full = _unshard([res.results[c]["out"] for c in range(NCORES)])
    return np.ascontiguousarray(full, dtype=np.float32), res


def kernel(**inputs):
    out, _ = _run(inputs, trace=False)
    return out


# revision 32
# speedup vs baseline: 1.1723x; 1.0229x over previous
"""Trainium2 Bass kernel for nn_FactorGraphGRU (N=8192, H=64, 8 NeuronCores).

Strategy (memory-bound): row-shard the outputs across 8 cores (1024 each).
Each core streams transposed adjacency shards once from HBM in bf16:

  posn  [N, 1024] bf16  host-built positive mask of node_adj^T (exact 0/1)
  eat   [N, 1024] bf16  edge_adj^T values (bf16 round ~0.4%, tolerance 2e-2)

Per 128-row block the tensor engine runs 4 matmul passes against a
stationary [h_hi | h_lo] bf16 tile (hi/lo split keeps the attention-score
exponents accurate): P (node mask), R=relu(eat), Nm=min(eat,0), count
(pos_e vs ones).  relu on ACT, min/is_gt on DVE (bf16 fast modes); the
GPSIMD engine is never used (its elementwise path measured ~20x slower).

All downstream algebra is folded into host-precomputed stationaries:
  - M = sum_h - h_i - P is eliminated (coefficients on P/h + bias consts)
  - hi/lo recombine is folded into every consumer stationary ([W; W])
The tail runs in the [64, ROWS] transposed layout (this toolchain cannot
encode matmul outputs at a non-zero PSUM base partition).  The GAT softmax
collapses to the two-value form: es = a_p*(W^T R) + a_m*(W^T Nm), with
Z = cp*(wp-wm) + (N-1)*wm from the streamed positive-count row.
"""

import numpy as np
from contextlib import ExitStack

N = 8192
H = 64
NCORES = 8
ROWS = N // NCORES        # 1024 output rows per core
JB = 128                  # contraction block (SBUF partitions)
NJB = N // JB             # 64
NB2 = N // (2 * JB)       # 32 fp8 DoubleRow blocks (256 rows each)
CHUNK = 512               # PSUM bank free size (f32)
NCH = ROWS // CHUNK       # 2
ALPHA = 0.2               # leaky relu slope
DEBUG_DUMP = False        # test hook: dump intermediates as extra outputs
USE_FAST_RECIP = True     # custom-DVE reciprocal (falls back to stock op)


# ---------------------------------------------------------------------------
# walrus workaround: this toolchain accepts at most ONE sync wait per
# instruction; Tile attaches several.  Rewrite the BIR so every extra wait
# rides on its own NoOp carrier right before the instruction.
# ---------------------------------------------------------------------------
def _split_multiwaits(nc):
    import bass_rust
    import concourse.mybir as mybir

    ctr = [0]

    def carrier(engine, wait):
        ctr[0] += 1
        nop = bass_rust.InstNoOp(name=f"WS-{ctr[0]}", engine=engine, ins=[], outs=[])
        nop.sync_info = mybir.SyncInfo(on_wait=[wait], on_update=[])
        return nop

    for fn in nc.m.functions:
        stack = list(fn.blocks)
        while stack:
            bb = stack.pop()
            stack.extend(getattr(bb, "blocks", []) or [])
            out = []
            changed = False
            for inst in bb.instructions:
                si = inst.sync_info
                waits = list(si.on_wait) if si is not None and si.on_wait else []
                if len(waits) > 1:
                    for w in waits[:-1]:
                        out.append(carrier(inst.engine, w))
                    si.on_wait = [waits[-1]]
                    changed = True
                out.append(inst)
            if changed:
                bb.instructions = out
    return nc


def _build_nc():
    import concourse.bass as bass
    import concourse.tile as tile
    from concourse import mybir

    F32 = mybir.dt.float32
    F32R = mybir.dt.float32r
    BF16 = mybir.dt.bfloat16
    FP8 = mybir.dt.float8e4
    AF = mybir.ActivationFunctionType
    OP = mybir.AluOpType

    nc = bass.Bass("TRN2", target_bir_lowering=False, debug=False,
                   num_devices=NCORES)

    # --- DRAM inputs (per-core shards via in_maps) ---
    comb = nc.dram_tensor("comb", [N // 2, 6 * ROWS], FP8, kind="ExternalInput").ap()
    h2p_d = nc.dram_tensor("h2p", [JB, N], BF16, kind="ExternalInput").ap()
    h8p_d = nc.dram_tensor("h8p", [JB, N], FP8, kind="ExternalInput").ap()
    ones8_d = nc.dram_tensor("ones8", [JB, 32], FP8, kind="ExternalInput").ap()
    hTp_d = nc.dram_tensor("hTp", [H, ROWS], F32, kind="ExternalInput").ap()
    hTpr_d = nc.dram_tensor("hTpr", [H, ROWS], F32R, kind="ExternalInput").ap()
    WeP_d = nc.dram_tensor("WeP", [2 * H, 3 * H], F32R, kind="ExternalInput").ap()
    Weh_d = nc.dram_tensor("Weh", [H, 4 * H], F32R, kind="ExternalInput").ap()
    WnX_d = nc.dram_tensor("WnX", [H, 3 * H], F32R, kind="ExternalInput").ap()
    Wnh_d = nc.dram_tensor("Wnh", [H, 3 * H], F32R, kind="ExternalInput").ap()
    be4_d = nc.dram_tensor("be4", [H, 4], F32, kind="ExternalInput").ap()
    bn4_d = nc.dram_tensor("bn4", [H, 4], F32, kind="ExternalInput").ap()
    Wg2_d = nc.dram_tensor("Wg2", [2 * H, H], F32R, kind="ExternalInput").ap()
    Wg2n_d = nc.dram_tensor("Wg2n", [2 * H, H], F32R, kind="ExternalInput").ap()
    vecsP_d = nc.dram_tensor("vecsP", [2 * H, 2], F32R, kind="ExternalInput").ap()
    vech_d = nc.dram_tensor("vech", [H, 2], F32R, kind="ExternalInput").ap()
    cbias_d = nc.dram_tensor("cbias", [1, 2], F32, kind="ExternalInput").ap()
    ones1_d = nc.dram_tensor("ones1", [1, H], F32R, kind="ExternalInput").ap()
    d_er_d = nc.dram_tensor("d_er", [1, ROWS], F32R, kind="ExternalInput").ap()
    d_nr_d = nc.dram_tensor("d_nr", [1, ROWS], F32R, kind="ExternalInput").ap()
    out = nc.dram_tensor("out", [H, ROWS], F32, kind="ExternalOutput").ap()
    dbg = {}
    if DEBUG_DUMP:
        for nm, sh in [("d_P", [2 * H, CHUNK]), ("d_ep", [1, ROWS]),
                       ("d_em", [1, ROWS]), ("d_cp", [1, ROWS]),
                       ("d_ap", [1, ROWS]), ("d_am", [1, ROWS]),
                       ("d_spos", [H, ROWS]), ("d_es", [H, ROWS]),
                       ("d_eo", [H, ROWS]), ("d_no", [H, ROWS])]:
            dbg[nm] = nc.dram_tensor(nm, sh, F32, kind="ExternalOutput").ap()

    with tile.TileContext(nc) as tc, ExitStack() as ctx:
        # --- pools ---
        cbp = ctx.enter_context(tc.tile_pool(name="cbp", bufs=4))
        var = ctx.enter_context(tc.tile_pool(name="var", bufs=4))
        small = ctx.enter_context(tc.tile_pool(name="small", bufs=1))
        work = ctx.enter_context(tc.tile_pool(name="work", bufs=1))
        psAcc = tc.alloc_tile_pool(name="psAcc", bufs=1, space="PSUM")

        # --- small persistent inputs ---
        def load_small(src, shape, name, dt=F32):
            t = small.tile(shape, dt, name=name)
            nc.sync.dma_start(t[:], src[:])
            return t

        # h2p/h8p loaded in 8 slices just-in-time (slice q covers blocks
        # [4q, 4q+4); q+1 is issued at block 4q+1, three blocks of lead)
        h2ps = small.tile([JB, N], BF16, name="h2ps")
        h8ps = small.tile([JB, N], FP8, name="h8ps")

        def load_hslices(q):
            qs = slice(q * (N // 8), (q + 1) * (N // 8))
            nc.sync.dma_start(h2ps[:, qs], h2p_d[:, qs])
            nc.sync.dma_start(h8ps[:, qs], h8p_d[:, qs])

        load_hslices(0)
        ones8 = load_small(ones8_d, [JB, 32], "ones8", FP8)

        # --- PSUM accumulators: 6x[128,512] + 2x[1,512] = 8 banks ---
        psP = [psAcc.tile([2 * H, CHUNK], F32, name=f"psP{c}", tag=f"psP{c}")
               for c in range(NCH)]
        psR = [psAcc.tile([2 * H, CHUNK], F32, name=f"psR{c}", tag=f"psR{c}")
               for c in range(NCH)]
        psN = [psAcc.tile([2 * H, CHUNK], F32, name=f"psN{c}", tag=f"psN{c}")
               for c in range(NCH)]
        psC = [psAcc.tile([16, CHUNK], F32, name=f"psC{c}", tag=f"psC{c}")
               for c in range(NCH)]

        # =================== stream: one pass over both adjacencies ========
        # node mask: bf16 128-row tiles; edge: fp8 DoubleRow 256-row blocks
        DR = mybir.MatmulPerfMode.DoubleRow
        for b in range(NB2):
            if b % 4 == 1 and b // 4 + 1 < 8:
                load_hslices(b // 4 + 1)
            # mask+edge bytes ride one DMA: [0:4096)=bf16 mask, [4096:6144)=fp8 edge
            cb_t = cbp.tile([JB, 6 * ROWS], FP8, name="cb_t")
            nc.sync.dma_start(cb_t[:], comb[b * JB:(b + 1) * JB, :])
            ea_t = cb_t[:, 4 * ROWS:6 * ROWS]
            relu_t = var.tile([JB, 2 * ROWS], FP8, name="relu_t")
            nc.scalar.activation(relu_t[:], ea_t[:], AF.Relu)
            pose_t = var.tile([JB, 2 * ROWS], FP8, name="pose_t")
            nc.vector.tensor_single_scalar(pose_t[:], ea_t[:], 0.0, OP.is_gt)

            pn_t = cb_t[:].bitcast(BF16)[:, 0:2 * ROWS]   # bf16 mask view
            for u in range(2):
                jb = 2 * b + u
                st = h2ps[:, jb * JB:(jb + 1) * JB]   # [128, 128] bf16 [hi|lo]
                for c in range(NCH):
                    cs = slice(u * ROWS + c * CHUNK, u * ROWS + (c + 1) * CHUNK)
                    nc.tensor.matmul(psP[c][:], st, pn_t[:, cs],
                                     start=(jb == 0), stop=(jb == NJB - 1))

            st8 = h8ps[:, b * 2 * JB:(b + 1) * 2 * JB].rearrange(
                "p (s m) -> p s m", s=2)          # [128, 2, 128] fp8
            on8 = ones8[:].rearrange("p (s m) -> p s m", s=2)  # [128, 2, 16]
            r3 = relu_t[:].rearrange("p (s m) -> p s m", s=2)  # [128, 2, 1024]
            m3 = ea_t[:].rearrange("p (s m) -> p s m", s=2)   # raw a: T pass
            g3 = pose_t[:].rearrange("p (s m) -> p s m", s=2)
            sa = (b == 0)
            so = (b == NB2 - 1)
            for c in range(NCH):
                cs = slice(c * CHUNK, (c + 1) * CHUNK)
                nc.tensor.matmul(psR[c][:], st8, r3[:, :, cs],
                                 start=sa, stop=so, perf_mode=DR)
                nc.tensor.matmul(psN[c][:], st8, m3[:, :, cs],
                                 start=sa, stop=so, perf_mode=DR)
                nc.tensor.matmul(psC[c][:], on8, g3[:, :, cs],
                                 start=sa, stop=so, perf_mode=DR)

        # tail-only params: loaded after the stream DMAs are queued so the
        # first adjacency tiles and h2p hit the DMA rings first
        hTp = load_small(hTp_d, [H, ROWS], "hTp")
        hTpr = load_small(hTpr_d, [H, ROWS], "hTpr", F32R)
        WeP = load_small(WeP_d, [2 * H, 3 * H], "WeP", F32R)
        Weh = load_small(Weh_d, [H, 4 * H], "Weh", F32R)
        WnX = load_small(WnX_d, [H, 3 * H], "WnX", F32R)
        Wnh = load_small(Wnh_d, [H, 3 * H], "Wnh", F32R)
        be4 = load_small(be4_d, [H, 4], "be4")
        bn4 = load_small(bn4_d, [H, 4], "bn4")
        Wg2 = load_small(Wg2_d, [2 * H, H], "Wg2", F32R)
        Wg2n = load_small(Wg2n_d, [2 * H, H], "Wg2n", F32R)
        vecsP = load_small(vecsP_d, [2 * H, 2], "vecsP", F32R)
        vech = load_small(vech_d, [H, 2], "vech", F32R)
        cbias = load_small(cbias_d, [1, 2], "cbias")
        ones1 = load_small(ones1_d, [1, H], "ones1", F32R)
        d_er = load_small(d_er_d, [1, ROWS], "d_er", F32R)
        d_nr = load_small(d_nr_d, [1, ROWS], "d_nr", F32R)

        # =================== tail ([64, ROWS] layout, chunk-pipelined) =====
        # copy accumulators to SBUF (P first: scores + edge GRU need it),
        # then release all 8 banks
        cpyP, cpyR, cpyN = [], [], []
        for c in range(NCH):
            tP = work.tile([2 * H, CHUNK], F32R, name=f"cpyP{c}")
            nc.scalar.copy(tP[:], psP[c][:])
            cpyP.append(tP)
        cp_row = work.tile([1, ROWS], F32, name="cp_row")
        for c in range(NCH):
            nc.scalar.copy(cp_row[:, c * CHUNK:(c + 1) * CHUNK], psC[c][0:1, :])
            tR = work.tile([2 * H, CHUNK], F32R, name=f"cpyR{c}")
            nc.scalar.copy(tR[:], psR[c][:])
            cpyR.append(tR)
            tN = work.tile([2 * H, CHUNK], F32R, name=f"cpyN{c}")
            nc.scalar.copy(tN[:], psN[c][:])
            cpyN.append(tN)
        psAcc.release()
        psG = ctx.enter_context(tc.tile_pool(name="psG", bufs=4, space="PSUM"))
        psRow = ctx.enter_context(tc.tile_pool(name="psRow", bufs=2, space="PSUM"))

        # persistent tail tiles (ops run per column chunk for pipelining);
        # row tiles share 7 rotating slots, GRU temps fold into gate tiles
        def wtile(name, shape=None, dt=F32, tag=None, bufs=1):
            return work.tile(shape or [H, ROWS], dt, name=name,
                             **({"tag": tag, "bufs": bufs} if tag else {}))

        def rtile(name):
            return work.tile([1, ROWS], F32, name=name, tag="row", bufs=6)

        ep_pre = rtile("ep_pre"); em_pre = rtile("em_pre")
        ep = rtile("ep"); em = rtile("em")
        m_row = rtile("m_row")
        wp = rtile("wp"); wm = rtile("wm")
        dw = rtile("dw"); tz = rtile("tz")
        z_row = rtile("z_row"); invz = rtile("invz")
        a_p = wtile("a_p", [1, ROWS], F32R); a_m = wtile("a_m", [1, ROWS], F32R)
        spos = wtile("spos"); sneg = wtile("sneg"); es_p = wtile("es_p", dt=F32R)
        ap_b = wtile("ap_b"); am_b = wtile("am_b")
        de_b = wtile("de_b"); dn_b = wtile("dn_b")
        gtiles = {}
        for nm in ("ge", "gn"):
            for t in ("s0", "s1", "s2", "hn"):
                gtiles[f"{nm}_{t}"] = wtile(f"{nm}_{t}")

        def score_mm(c, k, dst):
            cs = slice(c * CHUNK, (c + 1) * CHUNK)
            ps = psRow.tile([1, CHUNK], F32, name="ps_sc", tag="r")
            nc.tensor.matmul(ps[:], vecsP[:, k:k + 1], cpyP[c][:],
                             start=True, stop=False)
            nc.tensor.matmul(ps[:], vech[:, k:k + 1], hTpr[:, cs],
                             start=False, stop=True)
            nc.scalar.activation(dst[:, cs], ps[:], AF.Identity,
                                 bias=cbias[0:1, k:k + 1])

        def mm_copy(dst, c, lhsT, mov, name):
            cs = slice(c * CHUNK, (c + 1) * CHUNK)
            ps = psG.tile([H, CHUNK], F32, name=name, tag="g")
            nc.tensor.matmul(ps[:], lhsT, mov, start=True, stop=True)
            nc.scalar.copy(dst[:, cs], ps[:])

        def gru_chunk(c, nm, xs_P, xs_h, Wh, h_gates, hn_col, bias4):
            """One column-chunk of a GRU; result lands in gtiles[nm_s0]."""
            cs = slice(c * CHUNK, (c + 1) * CHUNK)
            for g, (fn, bcol) in enumerate((("sig", 0), ("sig", 1), ("id", 2))):
                gc = slice(g * H, (g + 1) * H)
                ps = psG.tile([H, CHUNK], F32, name=f"{nm}_g{g}", tag="g")
                mms = [(W_[:, gc], mov[c][:]) for mov, W_ in xs_P]
                mms += [(W_[:, gc], mov[:, cs]) for mov, W_ in xs_h]
                if g < h_gates:
                    mms.append((Wh[:, gc], hTpr[:, cs]))
                for k, (lh, mv) in enumerate(mms):
                    nc.tensor.matmul(ps[:], lh, mv, start=(k == 0),
                                     stop=(k == len(mms) - 1))
                nc.scalar.activation(
                    gtiles[f"{nm}_s{g}"][:, cs], ps[:],
                    AF.Sigmoid if fn == "sig" else AF.Identity,
                    bias=bias4[:, bcol:bcol + 1])
            ps = psG.tile([H, CHUNK], F32, name=f"{nm}_gh", tag="g")
            nc.tensor.matmul(ps[:], Wh[:, hn_col:hn_col + H], hTpr[:, cs],
                             start=True, stop=True)
            nc.scalar.activation(gtiles[f"{nm}_hn"][:, cs], ps[:], AF.Identity,
                                 bias=bias4[:, 3:4])
            r_s, z_s = gtiles[f"{nm}_s0"], gtiles[f"{nm}_s1"]
            ns, hn = gtiles[f"{nm}_s2"], gtiles[f"{nm}_hn"]
            # n = tanh(ns + r*hn); out = n + z*(h - n)   (all in place)
            nc.vector.tensor_tensor(hn[:, cs], r_s[:, cs], hn[:, cs], OP.mult)
            nc.vector.tensor_tensor(ns[:, cs], ns[:, cs], hn[:, cs], OP.add)
            nc.scalar.activation(ns[:, cs], ns[:, cs], AF.Tanh)
            nc.vector.tensor_tensor(r_s[:, cs], hTp[:, cs], ns[:, cs],
                                    OP.subtract)
            nc.vector.tensor_tensor(r_s[:, cs], z_s[:, cs], r_s[:, cs], OP.mult)
            nc.vector.tensor_tensor(r_s[:, cs], r_s[:, cs], ns[:, cs], OP.add)
            return r_s

        for c in range(NCH):
            cs = slice(c * CHUNK, (c + 1) * CHUNK)
            # scores -> leaky relu -> softmax weights -> Z -> a_p/a_m
            score_mm(c, 0, ep_pre)
            score_mm(c, 1, em_pre)
            nc.vector.scalar_tensor_tensor(ep[:, cs], ep_pre[:, cs], ALPHA,
                                           ep_pre[:, cs], OP.mult, OP.max)
            nc.vector.scalar_tensor_tensor(em[:, cs], em_pre[:, cs], ALPHA,
                                           em_pre[:, cs], OP.mult, OP.max)
            nc.vector.tensor_tensor(m_row[:, cs], ep[:, cs], em[:, cs], OP.max)
            nc.vector.tensor_tensor(wp[:, cs], ep[:, cs], m_row[:, cs],
                                    OP.subtract)
            nc.scalar.activation(wp[:, cs], wp[:, cs], AF.Exp)
            nc.vector.tensor_tensor(wm[:, cs], em[:, cs], m_row[:, cs],
                                    OP.subtract)
            nc.scalar.activation(wm[:, cs], wm[:, cs], AF.Exp)
            nc.vector.tensor_tensor(dw[:, cs], wp[:, cs], wm[:, cs],
                                    OP.subtract)
            nc.vector.tensor_tensor(tz[:, cs], dw[:, cs], cp_row[:, cs],
                                    OP.mult)
            nc.vector.scalar_tensor_tensor(z_row[:, cs], wm[:, cs],
                                           float(N - 1), tz[:, cs],
                                           OP.mult, OP.add)
            nc.vector.reciprocal(invz[:, cs], z_row[:, cs])
            nc.vector.tensor_tensor(a_p[:, cs], wp[:, cs], invz[:, cs],
                                    OP.mult)
            nc.vector.tensor_tensor(a_m[:, cs], wm[:, cs], invz[:, cs],
                                    OP.mult)
            # GAT output: es = (ap_b*spos) + (am_b*sneg), in place
            mm_copy(spos, c, Wg2[:], cpyR[c][:], "spos_ps")
            # sneg = W^T(T - R): psN streamed raw values, fold -R here
            psn_ = psG.tile([H, CHUNK], F32, name="sneg_ps", tag="g")
            nc.tensor.matmul(psn_[:], Wg2[:], cpyN[c][:], start=True, stop=False)
            nc.tensor.matmul(psn_[:], Wg2n[:], cpyR[c][:], start=False, stop=True)
            nc.scalar.copy(sneg[:, cs], psn_[:])
            mm_copy(ap_b, c, ones1[:], a_p[:, cs], "apb_ps")
            mm_copy(am_b, c, ones1[:], a_m[:, cs], "amb_ps")
            nc.vector.tensor_tensor(spos[:, cs], ap_b[:, cs], spos[:, cs],
                                    OP.mult)
            nc.vector.tensor_tensor(sneg[:, cs], am_b[:, cs], sneg[:, cs],
                                    OP.mult)
            nc.vector.tensor_tensor(es_p[:, cs], spos[:, cs], sneg[:, cs],
                                    OP.add)
            # GRUs
            eo = gru_chunk(c, "ge", [(cpyP, WeP)], [], Weh, 3, 3 * H, be4)
            no = gru_chunk(c, "gn", [], [(es_p, WnX)], Wnh, 2, 2 * H, bn4)
            # final mix (in place into the d bcasts) + store
            mm_copy(de_b, c, ones1[:], d_er[:, cs], "deb_ps")
            mm_copy(dn_b, c, ones1[:], d_nr[:, cs], "dnb_ps")
            nc.vector.tensor_tensor(de_b[:, cs], de_b[:, cs], eo[:, cs],
                                    OP.mult)
            nc.vector.tensor_tensor(dn_b[:, cs], dn_b[:, cs], no[:, cs],
                                    OP.mult)
            nc.vector.tensor_tensor(de_b[:, cs], de_b[:, cs], dn_b[:, cs],
                                    OP.add)
            nc.sync.dma_start(out[:, cs], de_b[:, cs])
        edge_out, node_out = gtiles["ge_s0"], gtiles["gn_s0"]
        fin = de_b

        if DEBUG_DUMP:
            for nm, t in [("d_P", cpyP[0]), ("d_ep", ep), ("d_em", em),
                          ("d_cp", cp_row), ("d_ap", a_p), ("d_am", a_m),
                          ("d_spos", spos), ("d_es", es_p),
                          ("d_eo", edge_out), ("d_no", node_out)]:
                nc.sync.dma_start(dbg[nm][:], t[:].bitcast(mybir.dt.float32))

    _split_multiwaits(nc)
    return nc


def _host_prep(inputs):
    import ml_dtypes

    BF = ml_dtypes.bfloat16
    h = np.ascontiguousarray(inputs["h"], dtype=np.float32)
    node_adj = np.asarray(inputs["node_adj"], dtype=np.float32)
    edge_adj = np.asarray(inputs["edge_adj"], dtype=np.float32)
    W_gat = np.asarray(inputs["W_gat"], dtype=np.float32)
    a_gat = np.asarray(inputs["a_gat"], dtype=np.float32)
    w_ih_e = np.asarray(inputs["w_ih_e"], dtype=np.float32)
    w_hh_e = np.asarray(inputs["w_hh_e"], dtype=np.float32)
    b_ih_e = np.asarray(inputs["b_ih_e"], dtype=np.float32)
    b_hh_e = np.asarray(inputs["b_hh_e"], dtype=np.float32)
    w_ih_n = np.asarray(inputs["w_ih_n"], dtype=np.float32)
    w_hh_n = np.asarray(inputs["w_hh_n"], dtype=np.float32)
    b_ih_n = np.asarray(inputs["b_ih_n"], dtype=np.float32)
    b_hh_n = np.asarray(inputs["b_hh_n"], dtype=np.float32)

    d_node = np.ascontiguousarray(np.diag(node_adj)).astype(np.float32)
    d_edge = np.ascontiguousarray(np.diag(edge_adj)).astype(np.float32)

    FP8 = ml_dtypes.float8_e4m3
    h_hi = h.astype(BF).astype(np.float32)
    h_lo = (h - h_hi).astype(BF).astype(np.float32)
    h8_hi = h.astype(FP8).astype(np.float32)
    h8_lo = (h - h8_hi).astype(FP8)
    sum_h = h.sum(axis=0, dtype=np.float64).astype(np.float32)    # [H]

    # h2p [128, N]: h2p[p, jb*128+m] = (m<64 ? h_hi : h_lo)[jb*128+p, m%64]
    hi3 = h_hi.reshape(NJB, JB, H).transpose(1, 0, 2)
    lo3 = h_lo.reshape(NJB, JB, H).transpose(1, 0, 2)
    h2p = np.concatenate([hi3, lo3], axis=2).reshape(JB, N).astype(BF)
    # h8p [128, N] fp8 for DoubleRow: (p, b*256 + s*128 + m) = h8[b*256+2p+s, m]
    h8cat = np.concatenate([h8_hi.astype(FP8).astype(np.float32),
                            h8_lo.astype(np.float32)], axis=1)      # [N, 128]
    h8p = (h8cat.reshape(NB2, JB, 2, 2 * H).transpose(1, 0, 2, 3)
           .reshape(JB, N)).astype(FP8)

    a1 = a_gat[0:H, 0]
    a2 = a_gat[H:2 * H, 0]
    Wa1 = W_gat @ a1
    Wa2 = W_gat @ a2

    def stack2(x):
        return np.ascontiguousarray(np.concatenate([x, x], axis=0),
                                    dtype=np.float32)

    vecsP = stack2(np.stack([Wa1 - Wa2, Wa2 - Wa1], axis=1))
    vech = np.ascontiguousarray(np.stack([-Wa2, -Wa1], axis=1), np.float32)
    cbias = np.array([[float(sum_h @ Wa2), float(sum_h @ Wa1)]], np.float32)

    wieP = np.ascontiguousarray(w_ih_e.T[0:H, :])       # [64, 192]
    wieM = np.ascontiguousarray(w_ih_e.T[H:2 * H, :])
    whhe = np.ascontiguousarray(w_hh_e.T)               # [64, 192]
    wihn = np.ascontiguousarray(w_ih_n.T)
    whhn = np.ascontiguousarray(w_hh_n.T)

    WeP = stack2(wieP - wieM)
    Weh = np.zeros((H, 4 * H), np.float32)
    Weh[:, 0:2 * H] = -wieM[:, 0:2 * H] + whhe[:, 0:2 * H]        # r|z
    Weh[:, 2 * H:3 * H] = -wieM[:, 2 * H:3 * H]                   # in
    Weh[:, 3 * H:4 * H] = whhe[:, 2 * H:3 * H]                    # hn
    WnX = np.ascontiguousarray(wihn)
    Wnh = np.zeros((H, 3 * H), np.float32)
    Wnh[:, 0:2 * H] = whhn[:, 0:2 * H]                            # r|z
    Wnh[:, 2 * H:3 * H] = whhn[:, 2 * H:3 * H]                    # hn

    be4 = np.zeros((H, 4), np.float32)
    be4[:, 0] = b_ih_e[0:H] + b_hh_e[0:H] + wieM[:, 0:H].T @ sum_h
    be4[:, 1] = (b_ih_e[H:2 * H] + b_hh_e[H:2 * H]
                 + wieM[:, H:2 * H].T @ sum_h)
    be4[:, 2] = b_ih_e[2 * H:3 * H] + wieM[:, 2 * H:3 * H].T @ sum_h
    be4[:, 3] = b_hh_e[2 * H:3 * H]
    bn4 = np.zeros((H, 4), np.float32)
    bn4[:, 0] = b_ih_n[0:H] + b_hh_n[0:H]
    bn4[:, 1] = b_ih_n[H:2 * H] + b_hh_n[H:2 * H]
    bn4[:, 2] = b_ih_n[2 * H:3 * H]
    bn4[:, 3] = b_hh_n[2 * H:3 * H]

    shared = {
        "h2p": h2p, "h8p": h8p,
        "ones8": np.ones((JB, 32), FP8),
        "WeP": WeP, "Weh": Weh, "WnX": WnX, "Wnh": Wnh,
        "be4": be4, "bn4": bn4,
        "Wg2": stack2(W_gat), "Wg2n": stack2(-W_gat), "vecsP": vecsP, "vech": vech, "cbias": cbias,
        "ones1": np.ones((1, H), np.float32),
    }

    nat_full = np.ascontiguousarray(node_adj.T)
    eat_full = np.ascontiguousarray(edge_adj.T)
    idx = np.arange(ROWS)
    in_maps = []
    for c in range(NCORES):
        sl = slice(c * ROWS, (c + 1) * ROWS)
        nat = nat_full[:, sl].copy()
        nat[c * ROWS + idx, idx] = 0.0
        eat = eat_full[:, sl].copy()
        eat[c * ROWS + idx, idx] = 0.0
        m = dict(shared)
        pn_mask = (nat > 0).astype(np.float32)
        pn_b = np.ascontiguousarray(
            pn_mask.reshape(NB2, 2, JB, ROWS).transpose(0, 2, 1, 3)
            .reshape(N // 2, 2 * ROWS)).astype(BF)
        ea_b = np.ascontiguousarray(eat.astype(FP8).reshape(N // 2, 2 * ROWS))
        m["comb"] = np.ascontiguousarray(np.concatenate(
            [pn_b.view(np.uint8), ea_b.view(np.uint8)], axis=1)).view(FP8)
        hTp = np.ascontiguousarray(h[sl].T)
        m["hTp"] = hTp
        m["hTpr"] = hTp
        m["d_er"] = d_edge[sl].reshape(1, ROWS).copy()
        m["d_nr"] = d_node[sl].reshape(1, ROWS).copy()
        in_maps.append(m)
    return in_maps


def _unshard(outs):
    full = np.empty((N, H), np.float32)
    for c in range(NCORES):
        full[c * ROWS:(c + 1) * ROWS, :] = outs[c].T   # [64, 1024] -> rows
    return full


def _run(inputs, trace=False, tmpdir=None):
    from concourse.bass_utils import run_bass_kernel_spmd

    in_maps = _host_prep(inputs)
    nc = _build_nc()
    res = run_bass_kernel_spmd(nc, in_maps, core_ids=list(range(NCORES)),
                               trace=trace, tmpdir=tmpdir)
    full = _unshard([res.results[c]["out"] for c in range(NCORES)])
    return np.ascontiguousarray(full, dtype=np.float32), res


def kernel(**inputs):
    out, _ = _run(inputs, trace=False)
    return out
